# revision 30
# baseline (speedup 1.0000x reference)
"""Trainium2 Bass kernel for nn_Encoder_Block (B=2,S=2048,E=1024,H=16,D=64,FE=4).

Sharding: 8 NeuronCores, no collectives. Cores 0-3 take batch 0, cores 4-7
batch 1; each core owns a 512-query slice and runs the full encoder block
for those queries (it loads all keys/values of its batch plus all weights).

Attention math (per head, per core; scores need ~20+ mantissa bits because the
reference multiplies logits by sqrt(S)=45.25, so bf16/fp16 single products
flip argmaxes of the near-one-hot softmax):
  q' = q @ (Wq^T Wk) folded on host; split q' = q_hi + q_lo, k = k_hi + k_lo
  in fp16 (11-bit mantissa each; products are exact in fp32 PSUM).
  pass1 [q,k]: s_hi = q_hi.k_hi -> row max via DVE tensor_tensor_reduce pairs
               over PSUM chunks -> -max as fp16 [128,QT]
  flip: one SBUF->SBUF DMA turns -max [128,QT] into the [1,T] aug row of qaug
        (query perm order j = r*QT + qt makes that DMA a linear copy)
  pass2 [k,q] per k-tile: psum = [k_hi;k_lo].[q_hi;q_hi] + [k_hi;ones].[q_lo;-m]
        = k.q_hi + k_hi.q_lo - max   (drops only k_lo.q_lo ~ 2^-24)
  exp via ACT (scale=sqrt(S)) -> attnT bf16 -> ov: [v|1].T @ attnT accumulated
  over k-tiles gives [ovT; Z]; 1/Z broadcast multiplies during the drain
  straight into the packed fc input (Wv folded into Wfc on host).
Then fc + residual + LN1, FFN1(+relu, bias), FFN2(+bias), residual + LN2.
v is host-pretiled with the ones column baked in; weight/value DMAs ride the
GPSIMD SWDGE queue to keep the HWDGE path clear for the latency-critical
attention streams.
"""
import os
import sys
import math
from contextlib import ExitStack

os.environ.setdefault("NEURON_RT_RESET_CORES", "1")
sys.path.insert(0, "/opt/trn_rl_repo")

import numpy as np
import concourse.bass as bass
import concourse.tile as tile
from concourse import mybir

F32 = mybir.dt.float32
F16 = mybir.dt.float16
BF16 = mybir.dt.bfloat16
AX = mybir.AxisListType.X
AF = mybir.ActivationFunctionType
OP = mybir.AluOpType

class Cfg:
    def __init__(self, S=2048, E=1024, H=16, D=64, FE=4, T=512, eps=1e-5):
        self.S, self.E, self.H, self.D, self.FE, self.T, self.eps = S, E, H, D, FE, T, eps
        assert D == 64 and E == H * D
        self.KT = S // 128            # k partition-tiles
        self.QT = T // 128            # q tiles (per core)
        self.ET = E // 128            # e tiles
        self.ZT = FE * E // 128       # ffn hidden tiles
        self.CH = min(512, S)         # k moving chunk for pass1
        self.NCH = S // self.CH
        self.EC = min(512, E)         # e moving chunk
        self.NEC = E // self.EC
        self.P2B = 2                  # pass-2 k-tiles per exp batch
        self.scale = math.sqrt(float(S))

    def perm(self):
        # pass-2 query order j <-> original query (j % QT)*128 + j // QT
        j = np.arange(self.T)
        return (j % self.QT) * 128 + j // self.QT


def _layernorm(nc, pool, x_ap, out_ap, g_b, b_b, eps_t, c, out_dtype=None):
    """LayerNorm over the free dim (E) of x_ap [128, E] -> out_ap.

    The affine (x - mu) * rstd runs on ACT (Identity: per-partition scale and
    bias APs, same table as Exp/Relu so no table reload); DVE keeps the stats
    and the per-free-element gamma/beta ops."""
    E = c.E
    nsub = (E + 511) // 512
    stats = pool.tile([128, nsub, 6], F32, tag="ln_stats")
    xr = x_ap.rearrange("p (n s) -> p n s", n=nsub)
    for i in range(nsub):
        nc.vector.bn_stats(stats[:, i, :], xr[:, i, :])
    mv = pool.tile([128, 2], F32, tag="ln_mv")
    nc.vector.bn_aggr(mv[:], stats[:])
    rstd = pool.tile([128, 1], F32, tag="ln_rstd")
    nc.scalar.activation(rstd[:], mv[:, 1:2], AF.Sqrt, bias=eps_t[:], scale=1.0)
    nc.vector.reciprocal(rstd[:], rstd[:])
    nmr = pool.tile([128, 1], F32, tag="ln_nmr")
    nc.vector.scalar_tensor_tensor(
        nmr[:], mv[:, 0:1], -1.0, rstd[:], OP.mult, OP.mult)
    t1 = pool.tile([128, E], F32, tag="ln_t1")
    nc.scalar.activation(t1[:], x_ap, AF.Identity, bias=nmr[:], scale=rstd[:])
    nc.vector.tensor_tensor(t1[:], t1[:], g_b[:], OP.mult)
    nc.vector.tensor_tensor(out_ap, t1[:], b_b[:], OP.add)


def build_nc(c: Cfg):
    """Build the single-core program (pure SPMD — all cores run this)."""
    nc = bass.Bass()
    S, E, H, D, T = c.S, c.E, c.H, c.D, c.T

    dp = nc.declare_dram_parameter
    khl_d = dp("khl", [H, 128, S], F16, isOutput=False)      # [1; k_hi^T; k_lo^T[:63]]
    qdup_d = dp("qdup", [H, 128, T], F16, isOutput=False)    # [0; q_hi; q_hi[:63]] perm
    qh_d = dp("qh", [H, 128, T], F16, isOutput=False)        # [0; q_hi^T; 0] orig order
    qlo_d = dp("qlo", [E, T], F16, isOutput=False)           # q_lo^T perm order
    qnat_d = dp("qnat", [T, E], F32, isOutput=False)         # queries rows (perm order)
    v_d = dp("vv", [H, 128, c.KT * 65], BF16, isOutput=False)  # pretiled v + ones col
    wfc_d = dp("wfc", [128, c.ET, E], BF16, isOutput=False)  # Wfc_v^T tiled
    bfc_d = dp("bfc", [1, E], BF16, isOutput=False)
    w1_d = dp("w1", [c.ZT, 128, E], BF16, isOutput=False)    # per zt: [e_in part, z cols]
    b1_d = dp("b1", [128, c.ZT], F32, isOutput=False)
    w2_d = dp("w2", [c.ZT, 128, E], BF16, isOutput=False)    # per zt: [z part, e cols]
    b2_d = dp("b2", [1, E], BF16, isOutput=False)
    g1_d = dp("g1", [1, E], F32, isOutput=False)
    be1_d = dp("be1", [1, E], F32, isOutput=False)
    g2_d = dp("g2", [1, E], F32, isOutput=False)
    be2_d = dp("be2", [1, E], F32, isOutput=False)
    out_d = dp("out", [T, E], F32, isOutput=True)            # perm rows

    with tile.TileContext(nc) as tc, ExitStack() as ctx:
        persist = ctx.enter_context(tc.tile_pool(name="persist", bufs=1))

        def bcast128(src_ap, nm, dtype=BF16):
            t = persist.tile([128, src_ap.shape[1]], dtype, name=nm, tag=nm)
            src_b = bass.AP(tensor=src_ap.tensor, offset=src_ap.offset,
                            ap=[[0, 128]] + list(src_ap.ap[1:]))
            nc.gpsimd.dma_start(t[:], src_b)
            return t

        g1_b = bcast128(g1_d[:], "g1b")
        be1_b = bcast128(be1_d[:], "be1b")
        g2_b = bcast128(g2_d[:], "g2b")
        be2_b = bcast128(be2_d[:], "be2b")

        eps_t = persist.tile([128, 1], F32)
        nc.vector.memset(eps_t[:], c.eps)

        ones_bf = persist.tile([1, 128], BF16)
        nc.vector.memset(ones_bf[:], 1.0)

        # staged via the GPSIMD SWDGE queue: first use is after attention, and
        # this keeps the HWDGE path clear for head 0's latency-critical streams
        wfc_t = persist.tile([128, c.ET, E], BF16)
        nc.gpsimd.dma_start(wfc_t[:], wfc_d[:])
        bfc_t = persist.tile([1, E], BF16)
        nc.gpsimd.dma_start(bfc_t[:], bfc_d[:])
        b1_t = persist.tile([128, c.ZT], F32)
        nc.gpsimd.dma_start(b1_t[:], b1_d[:])
        b2_t = persist.tile([1, E], BF16)
        nc.gpsimd.dma_start(b2_t[:], b2_d[:])

        ovT_pack = persist.tile([128, c.ET, T], BF16)
        h_sb = persist.tile([128, c.QT, E], F32)
        hT_bf = persist.tile([128, c.ET, T], BF16)
        z1rel = persist.tile([128, c.ZT, T], BF16)

        # ---- flat pools; PSUM = three shared rings (2+4+2 banks), reused
        # across attention/fc/FFN so phases overlap with no scope barriers ----
        khl_p = ctx.enter_context(tc.tile_pool(name="khl", bufs=3))
        qd_p = ctx.enter_context(tc.tile_pool(name="qd", bufs=3))
        qh_p = ctx.enter_context(tc.tile_pool(name="qh", bufs=3))
        qaug_p = ctx.enter_context(tc.tile_pool(name="qaug", bufs=3))
        vv_p = ctx.enter_context(tc.tile_pool(name="vv", bufs=3))
        sm_p = ctx.enter_context(tc.tile_pool(name="sm", bufs=2))
        scr_p = ctx.enter_context(tc.tile_pool(name="scr", bufs=1))
        attn_p = ctx.enter_context(tc.tile_pool(name="attn", bufs=2))
        hb_p = ctx.enter_context(tc.tile_pool(name="hb", bufs=1))
        zi_p = ctx.enter_context(tc.tile_pool(name="zi", bufs=2))
        zdr_p = ctx.enter_context(tc.tile_pool(name="zdr", bufs=2, space="DRAM"))
        st_p = ctx.enter_context(tc.tile_pool(name="st", bufs=2))
        w_p = ctx.enter_context(tc.tile_pool(name="wstream", bufs=4))
        r1_ps = ctx.enter_context(tc.tile_pool(name="r1_ps", bufs=2, space="PSUM"))
        r2_ps = ctx.enter_context(tc.tile_pool(name="r2_ps", bufs=2, space="PSUM"))
        r3_ps = ctx.enter_context(tc.tile_pool(name="r3_ps", bufs=2, space="PSUM"))

        # =================== ATTENTION ===================
        # khl layout: [ones(1); k_hi(64); k_lo(0:63)] so rows 0:65 double as
        # the aug matmul's lhsT (k_lo dim 63 is dropped: ~0.007 nats of noise).
        # qdup: [zeros(1); q_hi(64); q_hi(0:63)]; qaug: [-m(1); q_lo(64)].
        def pass1(h):
            khl_t = khl_p.tile([128, S], F16, tag="khl")
            nc.sync.dma_start(khl_t[:], khl_d[h])
            qdup_t = qd_p.tile([128, T], F16, tag="qdup")
            nc.sync.dma_start(qdup_t[:], qdup_d[h])
            qh_t = qh_p.tile([128, T], F16, tag="qh")
            nc.sync.dma_start(qh_t[:], qh_d[h])
            qaug_t = qaug_p.tile([65, T], F16, tag="qaug")
            nc.sync.dma_start(qaug_t[1:65, :], qlo_d[h * D:(h + 1) * D, :])
            vaug = vv_p.tile([128, c.KT * 65], BF16, tag="vaug")
            nc.gpsimd.dma_start(vaug[:], v_d[h])

            m_neg = sm_p.tile([128, c.QT], F16, tag="mneg")
            for qt in range(c.QT):
                mtmp = sm_p.tile([128, c.NCH], F32, tag="mtmp")
                for j in range(c.NCH):
                    ps = r1_ps.tile([128, c.CH], F32, tag="R1")
                    nc.tensor.matmul(
                        ps[:], qh_t[:, qt * 128:(qt + 1) * 128],
                        khl_t[:, j * c.CH:(j + 1) * c.CH],
                        start=True, stop=True)
                    nc.vector.reduce_max(mtmp[:, j:j + 1], ps[:], axis=AX)
                nc.vector.reduce_max(m_neg[:, qt:qt + 1], mtmp[:],
                                     axis=AX, negate=True)
            # flip -max [128,QT] -> qaug row 0 [1,T] (perm order)
            nc.sync.dma_start(qaug_t[0:1, :], m_neg[:])
            return khl_t, qdup_t, qaug_t, vaug

        def pass2(h, tiles):
            khl_t, qdup_t, qaug_t, vaug = tiles
            ovp = r3_ps.tile([65, T], F32, tag="R3")
            for tb in range(0, c.KT, c.P2B):
                p2 = r2_ps.tile([128, c.P2B, T], F32, tag="R2")
                for ti in range(c.P2B):
                    t = tb + ti
                    tsl = slice(t * 128, (t + 1) * 128)
                    nc.tensor.matmul(p2[:, ti, :], khl_t[:, tsl],
                                     qdup_t[:], start=True, stop=False)
                    nc.tensor.matmul(p2[:, ti, :], khl_t[:65, tsl],
                                     qaug_t[:], start=False, stop=True)
                attnT = attn_p.tile([128, c.P2B, T], BF16, tag="attnT")
                nc.scalar.activation(attnT[:], p2[:],
                                     AF.Exp, bias=0.0, scale=c.scale)
                for ti in range(c.P2B):
                    t = tb + ti
                    nc.tensor.matmul(
                        ovp[:], vaug[:, t * 65:(t + 1) * 65], attnT[:, ti, :],
                        start=(t == 0), stop=(t == c.KT - 1),
                        skip_group_check=True)

            # 1/Z broadcast and drain into packed fc input
            zrow = zi_p.tile([1, T], F32, tag="zrow")
            nc.vector.reciprocal(zrow[:], ovp[64:65, :])
            # ACT copy releases the ov PSUM ring ~4us earlier than the
            # zinv bounce; the scale then reads the SBUF copy
            ovcp = zi_p.tile([64, T], F32, tag="ovcp")
            nc.scalar.copy(ovcp[:], ovp[:64, :])
            zdr = zdr_p.tile([1, T], F32, tag="zdr")
            nc.sync.dma_start(zdr[:], zrow[:])
            zinv_b = zi_p.tile([64, T], F32, tag="zinv")
            zsrc = zdr[:]
            nc.sync.dma_start(
                zinv_b[:],
                bass.AP(tensor=zsrc.tensor, offset=zsrc.offset,
                        ap=[[0, 64]] + list(zsrc.ap[1:])))
            po = (h % 2) * 64
            nc.vector.scalar_tensor_tensor(
                ovT_pack[po:po + 64, h // 2, :], ovcp[:], 1.0, zinv_b[:],
                OP.bypass, OP.mult)

        # software pipeline: pass1 of head h+1 issues before pass2 of head
        # h, so PE never waits on the max->flip->qaug latency chain
        staged = pass1(0)
        for h in range(H):
            nxt = pass1(h + 1) if h + 1 < H else None
            pass2(h, staged)
            staged = nxt

        # =================== FC + LN1 + transpose(h) ===================
        # fc accumulators ride the R1 ring, so fc overlaps the attention drain
        for qt in range(c.QT):
            qsl = slice(qt * 128, (qt + 1) * 128)
            hpre = st_p.tile([128, E], F32, tag="hpre")
            nc.sync.dma_start(hpre[:], qnat_d[qsl, :])
            for ec in range(c.NEC):
                esl = slice(ec * c.EC, (ec + 1) * c.EC)
                aps = r1_ps.tile([128, c.EC], F32, tag="R1")
                for dt in range(c.ET):
                    nc.tensor.matmul(aps[:], ovT_pack[:, dt, qsl],
                                     wfc_t[:, dt, esl],
                                     start=(dt == 0), stop=False)
                nc.tensor.matmul(aps[:], ones_bf[:, :128], bfc_t[:, esl],
                                 start=False, stop=True)
                nc.vector.scalar_tensor_tensor(
                    hpre[:, esl], aps[:], 1.0, hpre[:, esl],
                    OP.bypass, OP.add)

            _layernorm(nc, st_p, hpre[:], h_sb[:, qt, :], g1_b, be1_b, eps_t, c)
            hbf = hb_p.tile([128, E], BF16, tag="hbf")
            nc.scalar.copy(hbf[:], h_sb[:, qt, :])
            for et in range(c.ET):
                nc.sync.dma_start(hT_bf[:, et, qsl],
                                  hbf[:, et * 128:(et + 1) * 128],
                                  transpose=True)

        # =================== FFN1 (zt pairs on the R2 ring) ===================
        for zp in range(c.ZT // 2):
            zps = r2_ps.tile([128, 2, T], F32, tag="R2")
            for i in range(2):
                zt = zp * 2 + i
                w1t = w_p.tile([128, E], BF16, tag="w1t")
                nc.gpsimd.dma_start(w1t[:], w1_d[zt, :, :])
                for half in range(2):
                    hsl = slice(half * (T // 2), (half + 1) * (T // 2))
                    for et in range(c.ET):
                        nc.tensor.matmul(zps[:, i, hsl],
                                         w1t[:, et * 128:(et + 1) * 128],
                                         hT_bf[:, et, hsl],
                                         start=(et == 0), stop=(et == c.ET - 1),
                                         skip_group_check=True)
            for i in range(2):
                zt = zp * 2 + i
                nc.scalar.activation(z1rel[:, zt, :], zps[:, i, :], AF.Relu,
                                     bias=b1_t[:, zt:zt + 1], scale=1.0)

        # ======== FFN2 + LN2, query-pair-serial so LN2 inlines ========
        # w2 is streamed once per query pair (re-read 2x, ~8MB extra DMA)
        # so each pair finishes early enough for its LN2 to overlap the next
        for qp in range(c.QT // 2):
            x_tiles = [r2_ps.tile([128, 2, c.EC], F32, tag="R2",
                                  name=f"x2_{qp}_{qi}")
                       for qi in range(2)]
            for zt in range(c.ZT):
                w2t = w_p.tile([128, E], BF16, tag="w2t")
                nc.gpsimd.dma_start(w2t[:], w2_d[zt, :, :])
                for qi in range(2):
                    qt = qp * 2 + qi
                    qsl = slice(qt * 128, (qt + 1) * 128)
                    for ec in range(c.NEC):
                        esl = slice(ec * c.EC, (ec + 1) * c.EC)
                        nc.tensor.matmul(
                            x_tiles[qi][:, ec, :], z1rel[:, zt, qsl],
                            w2t[:, esl], start=(zt == 0), stop=False,
                            skip_group_check=True)
            for qi in range(2):
                qt = qp * 2 + qi
                qsl = slice(qt * 128, (qt + 1) * 128)
                for ec in range(c.NEC):
                    esl = slice(ec * c.EC, (ec + 1) * c.EC)
                    nc.tensor.matmul(x_tiles[qi][:, ec, :], ones_bf[:, :128],
                                     b2_t[:, esl], start=False, stop=True,
                                     skip_group_check=True)
                xacc = st_p.tile([128, E], F32, tag="hpre")
                nc.vector.scalar_tensor_tensor(
                    xacc[:], x_tiles[qi][:].rearrange("p a b -> p (a b)"), 1.0,
                    h_sb[:, qt, :], OP.bypass, OP.add)
                outt = st_p.tile([128, E], F32, tag="ln_t1")
                _layernorm(nc, st_p, xacc[:], outt[:], g2_b, be2_b, eps_t, c)
                nc.sync.dma_start(out_d[qsl, :], outt[:])

    return nc


def _split_waits(nc, maxw=1):
    """walrus in this toolchain only accepts 1 sync-wait per instruction on
    several formats; move excess waits onto preceding same-engine NoOps."""
    ctr = 0
    for f in nc.m.functions:
        for bb in f.blocks:
            out = []
            for inst in bb.instructions:
                si = getattr(inst, "sync_info", None)
                if si is not None and si.on_wait and len(si.on_wait) > maxw:
                    waits = list(si.on_wait)
                    head, tail = waits[:-maxw], waits[-maxw:]
                    for i in range(0, len(head), maxw):
                        ctr += 1
                        out.append(mybir.InstNoOp(
                            name=f"waitsplit_{ctr}", engine=inst.engine,
                            ins=[], outs=[],
                            sync_info=mybir.SyncInfo(
                                on_wait=list(head[i:i + maxw]), on_update=[]),
                        ))
                    si.on_wait = tail
                out.append(inst)
            bb.instructions[:] = out


# ======================= host side =======================

def host_prep(c: Cfg, inputs, core):
    """Build the per-core input map (numpy only)."""
    B = inputs["queries"].shape[0]
    cores_per_batch = 8 // B if B <= 8 else 1
    b = core // cores_per_batch
    slot = core % cores_per_batch
    T = c.T
    perm = c.perm()

    q = np.asarray(inputs["queries"][b], np.float32)       # [S, E]
    k = np.asarray(inputs["keys"][b], np.float32)
    v = np.asarray(inputs["values"][b], np.float32)
    qs = q[slot * T:(slot + 1) * T]                        # [T, E]

    Wq = np.asarray(inputs["Wq"], np.float64)
    Wk = np.asarray(inputs["Wk"], np.float64)
    Wv = np.asarray(inputs["Wv"], np.float64)
    Wfc = np.asarray(inputs["Wfc"], np.float64)            # [E, E]
    W1 = np.asarray(inputs["W1"], np.float64)              # [FE*E, E]
    W2 = np.asarray(inputs["W2"], np.float64)              # [E, FE*E]

    # fold Wq/Wk into the queries: q'_h = q_h @ (Wq.T @ Wk); scores = q' @ k^T
    A_mid = Wq.T @ Wk
    E_, H_, D_ = c.E, c.H, c.D
    qp = np.empty((T, E_), np.float64)
    for h in range(H_):
        qp[:, h * D_:(h + 1) * D_] = qs[:, h * D_:(h + 1) * D_].astype(np.float64) @ A_mid
    qp = qp.astype(np.float32)

    q_hi = qp.astype(np.float16)
    q_lo = (qp - q_hi.astype(np.float32)).astype(np.float16)
    k_hi = k.astype(np.float16)
    k_lo = (k - k_hi.astype(np.float32)).astype(np.float16)

    # khl row 0 doubles as the aug-matmul ones row; k_lo dim 63 is dropped
    khl = np.empty((c.H, 128, c.S), np.float16)
    for h in range(H_):
        khl[h, 0] = 1.0
        khl[h, 1:65] = k_hi[:, h * D_:(h + 1) * D_].T
        khl[h, 65:] = k_lo[:, h * D_:h * D_ + 63].T

    # pass1 lhsT padded to 128 rows: zeros align with khl's ones/k_lo rows
    qh_pad = np.zeros((c.H, 128, T), np.float16)
    for h in range(H_):
        qh_pad[h, 1:65] = q_hi[:, h * D_:(h + 1) * D_].T
    qhp = q_hi[perm]                                       # [T, E] perm order
    qdup = np.empty((c.H, 128, T), np.float16)
    for h in range(H_):
        qdup[h, 0] = 0.0
        qdup[h, 1:65] = qhp[:, h * D_:(h + 1) * D_].T
        qdup[h, 65:] = qhp[:, h * D_:h * D_ + 63].T
    qlo_ship = np.ascontiguousarray(q_lo[perm].T)          # [E, T] perm order

    # pretiled v with ones column: v_prep[h, p, t*65+d] = v[t*128+p, h*64+d]
    vt = v.reshape(c.KT, 128, c.H, c.D).astype(ml_bf16())  # [t, p, h, d]
    v_prep = np.ones((c.H, 128, c.KT, 65), ml_bf16())
    v_prep[:, :, :, :64] = vt.transpose(2, 1, 0, 3)
    v_prep = v_prep.reshape(c.H, 128, c.KT * 65)

    # Wfc_v[e, h*64+d] = sum_dd Wfc[e, h*64+dd] * Wv[dd, d]
    E, H, D = c.E, c.H, c.D
    wfcv = np.empty((E, E), np.float64)
    for h in range(H):
        wfcv[:, h * D:(h + 1) * D] = Wfc[:, h * D:(h + 1) * D] @ Wv
    # rhs tiles: wfc_prep[p, dt, e] = Wfc_v[e, dt*128+p]
    wfc_prep = np.ascontiguousarray(
        wfcv.T.reshape(c.ET, 128, E).transpose(1, 0, 2)).astype(ml_bf16())

    # w1_prep[zt, p, et*128 + z] = W1[zt*128+z, et*128+p]
    w1r = W1.reshape(c.ZT, 128, c.ET, 128)                 # [zt, z, et, p]
    w1_prep = np.ascontiguousarray(
        w1r.transpose(0, 3, 2, 1).reshape(c.ZT, 128, E)).astype(ml_bf16())

    # w2_prep[zt, p, e] = W2[e, zt*128+p]
    w2r = W2.T.reshape(c.ZT, 128, E)                       # [zt, p, e]
    w2_prep = np.ascontiguousarray(w2r).astype(ml_bf16())

    b1 = np.asarray(inputs["b1"], np.float32)
    b1_prep = np.ascontiguousarray(b1.reshape(c.ZT, 128).T)  # [128, ZT]

    ret = {
        "khl": khl,
        "qdup": qdup,
        "qh": qh_pad,
        "qlo": qlo_ship,
        "qnat": np.ascontiguousarray(qs[perm]),
        "vv": v_prep,
        "wfc": wfc_prep,
        "bfc": np.asarray(inputs["bfc"], np.float32)[None, :].astype(ml_bf16()),
        "w1": w1_prep,
        "b1": b1_prep,
        "w2": w2_prep,
        "b2": np.asarray(inputs["b2"], np.float32)[None, :].astype(ml_bf16()),
        "g1": np.asarray(inputs["ln1_g"], np.float32)[None, :],
        "be1": np.asarray(inputs["ln1_b"], np.float32)[None, :],
        "g2": np.asarray(inputs["ln2_g"], np.float32)[None, :],
        "be2": np.asarray(inputs["ln2_b"], np.float32)[None, :],
    }
    return ret


def ml_bf16():
    import ml_dtypes
    return ml_dtypes.bfloat16


_CACHE = {}


def kernel(**inputs):
    """Full-input entry point: shard across 8 cores, run, gather."""
    c = Cfg()
    B, S, E = inputs["queries"].shape
    assert (B, S, E) == (2, c.S, c.E), (B, S, E)

    if "nc" not in _CACHE:
        nc = build_nc(c)
        _split_waits(nc)   # walrus wait-slot workaround (compile path only)
        _CACHE["nc"] = nc
    nc = _CACHE["nc"]

    in_maps = [host_prep(c, inputs, core) for core in range(8)]

    from concourse.bass_utils import run_bass_kernel_spmd
    res = run_bass_kernel_spmd(nc, in_maps, core_ids=list(range(8)))

    perm = c.perm()
    out = np.empty((B, S, E), np.float32)
    cores_per_batch = 4
    for core in range(8):
        b = core // cores_per_batch
        slot = core % cores_per_batch
        block = np.empty((c.T, E), np.float32)
        block[perm] = res.results[core]["out"]
        out[b, slot * c.T:(slot + 1) * c.T] = block
    return out


# revision 35
# speedup vs baseline: 1.0103x; 1.0103x over previous
"""Trainium2 Bass kernel for nn_Encoder_Block (B=2,S=2048,E=1024,H=16,D=64,FE=4).

Sharding: 8 NeuronCores, no collectives. Cores 0-3 take batch 0, cores 4-7
batch 1; each core owns a 512-query slice and runs the full encoder block
for those queries (it loads all keys/values of its batch plus all weights).

Attention math (per head, per core; scores need ~20+ mantissa bits because the
reference multiplies logits by sqrt(S)=45.25, so bf16/fp16 single products
flip argmaxes of the near-one-hot softmax):
  q' = q @ (Wq^T Wk) folded on host; split q' = q_hi + q_lo, k = k_hi + k_lo
  in fp16 (11-bit mantissa each; products are exact in fp32 PSUM).
  pass1 [q,k]: s_hi = q_hi.k_hi -> row max via DVE tensor_tensor_reduce pairs
               over PSUM chunks -> -max as fp16 [128,QT]
  flip: one SBUF->SBUF DMA turns -max [128,QT] into the [1,T] aug row of qaug
        (query perm order j = r*QT + qt makes that DMA a linear copy)
  pass2 [k,q] per k-tile: psum = [k_hi;k_lo].[q_hi;q_hi] + [k_hi;ones].[q_lo;-m]
        = k.q_hi + k_hi.q_lo - max   (drops only k_lo.q_lo ~ 2^-24)
  exp via ACT (scale=sqrt(S)) -> attnT bf16 -> ov: [v|1].T @ attnT accumulated
  over k-tiles gives [ovT; Z]; 1/Z broadcast multiplies during the drain
  straight into the packed fc input (Wv folded into Wfc on host).
Then fc + residual + LN1, FFN1(+relu, bias), FFN2(+bias), residual + LN2.
v is host-pretiled with the ones column baked in; weight/value DMAs ride the
GPSIMD SWDGE queue to keep the HWDGE path clear for the latency-critical
attention streams.
"""
import os
import sys
import math
from contextlib import ExitStack

os.environ.setdefault("NEURON_RT_RESET_CORES", "1")
sys.path.insert(0, "/opt/trn_rl_repo")

import numpy as np
import concourse.bass as bass
import concourse.tile as tile
from concourse import mybir

F32 = mybir.dt.float32
F16 = mybir.dt.float16
BF16 = mybir.dt.bfloat16
AX = mybir.AxisListType.X
AF = mybir.ActivationFunctionType
OP = mybir.AluOpType

class Cfg:
    def __init__(self, S=2048, E=1024, H=16, D=64, FE=4, T=512, eps=1e-5):
        self.S, self.E, self.H, self.D, self.FE, self.T, self.eps = S, E, H, D, FE, T, eps
        assert D == 64 and E == H * D
        self.KT = S // 128            # k partition-tiles
        self.QT = T // 128            # q tiles (per core)
        self.ET = E // 128            # e tiles
        self.ZT = FE * E // 128       # ffn hidden tiles
        self.CH = min(512, S)         # k moving chunk for pass1
        self.NCH = S // self.CH
        self.EC = min(512, E)         # e moving chunk
        self.NEC = E // self.EC
        self.P2B = 2                  # pass-2 k-tiles per exp batch
        self.scale = math.sqrt(float(S))

    def perm(self):
        # pass-2 query order j <-> original query (j % QT)*128 + j // QT
        j = np.arange(self.T)
        return (j % self.QT) * 128 + j // self.QT


def _layernorm(nc, pool, x_ap, out_ap, g_b, b_b, eps_t, c,
               identity=False, bf16_out=None):
    """LayerNorm over the free dim (E) of x_ap [128, E] -> out_ap.

    The affine (x - mu) * rstd runs on ACT (Identity: per-partition scale and
    bias APs, same table as Exp/Relu so no table reload); DVE keeps the stats
    and the per-free-element gamma/beta ops. With identity=True (gamma==1,
    beta==0, detected from the actual inputs at build time) the gamma/beta
    DVE ops are skipped and ACT writes out_ap (and optionally a bf16 twin)
    directly."""
    E = c.E
    nsub = (E + 511) // 512
    stats = pool.tile([128, nsub, 6], F32, tag="ln_stats")
    xr = x_ap.rearrange("p (n s) -> p n s", n=nsub)
    for i in range(nsub):
        nc.vector.bn_stats(stats[:, i, :], xr[:, i, :])
    mv = pool.tile([128, 2], F32, tag="ln_mv")
    nc.vector.bn_aggr(mv[:], stats[:])
    rstd = pool.tile([128, 1], F32, tag="ln_rstd")
    nc.scalar.activation(rstd[:], mv[:, 1:2], AF.Sqrt, bias=eps_t[:], scale=1.0)
    nc.vector.reciprocal(rstd[:], rstd[:])
    nmr = pool.tile([128, 1], F32, tag="ln_nmr")
    nc.vector.scalar_tensor_tensor(
        nmr[:], mv[:, 0:1], -1.0, rstd[:], OP.mult, OP.mult)
    if identity:
        if bf16_out is not None:
            nc.scalar.activation(bf16_out, x_ap, AF.Identity,
                                 bias=nmr[:], scale=rstd[:])
            nc.scalar.activation(out_ap, x_ap, AF.Identity,
                                 bias=nmr[:], scale=rstd[:])
            return
        half = E // 2
        for i in range(2):
            nc.scalar.activation(out_ap[:, i * half:(i + 1) * half],
                                 x_ap[:, i * half:(i + 1) * half],
                                 AF.Identity, bias=nmr[:], scale=rstd[:])
        return
    t1 = pool.tile([128, E], F32, tag="ln_t1")
    nc.scalar.activation(t1[:], x_ap, AF.Identity, bias=nmr[:], scale=rstd[:])
    if bf16_out is not None:
        nc.vector.tensor_tensor(t1[:], t1[:], g_b[:], OP.mult)
        nc.vector.tensor_tensor(out_ap, t1[:], b_b[:], OP.add)
        nc.scalar.copy(bf16_out, out_ap)
        return
    nc.vector.tensor_tensor(t1[:], t1[:], g_b[:], OP.mult)
    nc.vector.tensor_tensor(out_ap, t1[:], b_b[:], OP.add)


def build_nc(c: Cfg, ln1_id=False, ln2_id=False):
    """Build the single-core program (pure SPMD — all cores run this)."""
    nc = bass.Bass()
    S, E, H, D, T = c.S, c.E, c.H, c.D, c.T

    dp = nc.declare_dram_parameter
    khl_d = dp("khl", [H, 128, S], F16, isOutput=False)      # [1; k_hi^T; k_lo^T[:63]]
    qdup_d = dp("qdup", [H, 128, T], F16, isOutput=False)    # [0; q_hi; q_hi[:63]] perm
    qh_d = dp("qh", [H, 128, T], F16, isOutput=False)        # [0; q_hi^T; 0] orig order
    qlo_d = dp("qlo", [E, T], F16, isOutput=False)           # q_lo^T perm order
    qnat_d = dp("qnat", [T, E], F32, isOutput=False)         # queries rows (perm order)
    v_d = dp("vv", [H, 128, c.KT * 65], BF16, isOutput=False)  # pretiled v + ones col
    wfc_d = dp("wfc", [128, c.ET, E], BF16, isOutput=False)  # Wfc_v^T tiled
    w1_d = dp("w1", [c.ZT, 128, E], BF16, isOutput=False)    # per zt: [e_in part, z cols]
    b1_d = dp("b1", [128, c.ZT], F32, isOutput=False)
    w2_d = dp("w2", [c.ZT, 128, E], BF16, isOutput=False)    # per zt: [z part, e cols]
    b2_d = dp("b2", [1, E], BF16, isOutput=False)
    g1_d = dp("g1", [1, E], F32, isOutput=False)
    be1_d = dp("be1", [1, E], F32, isOutput=False)
    g2_d = dp("g2", [1, E], F32, isOutput=False)
    be2_d = dp("be2", [1, E], F32, isOutput=False)
    out_d = dp("out", [T, E], F32, isOutput=True)            # perm rows

    with tile.TileContext(nc) as tc, ExitStack() as ctx:
        persist = ctx.enter_context(tc.tile_pool(name="persist", bufs=1))

        def bcast128(src_ap, nm, dtype=BF16):
            t = persist.tile([128, src_ap.shape[1]], dtype, name=nm, tag=nm)
            src_b = bass.AP(tensor=src_ap.tensor, offset=src_ap.offset,
                            ap=[[0, 128]] + list(src_ap.ap[1:]))
            nc.gpsimd.dma_start(t[:], src_b)
            return t

        g1_b = bcast128(g1_d[:], "g1b")
        be1_b = bcast128(be1_d[:], "be1b")
        g2_b = bcast128(g2_d[:], "g2b")
        be2_b = bcast128(be2_d[:], "be2b")

        eps_t = persist.tile([128, 1], F32)
        nc.vector.memset(eps_t[:], c.eps)

        ones_bf = persist.tile([1, 128], BF16)
        nc.vector.memset(ones_bf[:], 1.0)

        # staged via the GPSIMD SWDGE queue: first use is after attention, and
        # this keeps the HWDGE path clear for head 0's latency-critical streams
        wfc_t = persist.tile([128, c.ET, E], BF16)
        nc.gpsimd.dma_start(wfc_t[:], wfc_d[:])
        b1_t = persist.tile([128, c.ZT], F32)
        nc.gpsimd.dma_start(b1_t[:], b1_d[:])
        b2_t = persist.tile([1, E], BF16)
        nc.gpsimd.dma_start(b2_t[:], b2_d[:])

        ovT_pack = persist.tile([128, c.ET, T], BF16)
        h_sb = persist.tile([128, c.QT, E], F32)
        hT_bf = persist.tile([128, c.ET, T], BF16)
        z1rel = persist.tile([128, c.ZT, T], BF16)

        # ---- flat pools; PSUM = three shared rings (2+4+2 banks), reused
        # across attention/fc/FFN so phases overlap with no scope barriers ----
        khl_p = ctx.enter_context(tc.tile_pool(name="khl", bufs=3))
        qd_p = ctx.enter_context(tc.tile_pool(name="qd", bufs=3))
        qh_p = ctx.enter_context(tc.tile_pool(name="qh", bufs=3))
        qaug_p = ctx.enter_context(tc.tile_pool(name="qaug", bufs=3))
        vv_p = ctx.enter_context(tc.tile_pool(name="vv", bufs=3))
        sm_p = ctx.enter_context(tc.tile_pool(name="sm", bufs=3))
        attn_p = ctx.enter_context(tc.tile_pool(name="attn", bufs=4))
        hb_p = ctx.enter_context(tc.tile_pool(name="hb", bufs=1))
        zi_p = ctx.enter_context(tc.tile_pool(name="zi", bufs=3))
        zdr_p = ctx.enter_context(tc.tile_pool(name="zdr", bufs=2, space="DRAM"))
        st_p = ctx.enter_context(tc.tile_pool(name="st", bufs=3))
        w_p = ctx.enter_context(tc.tile_pool(name="wstream", bufs=6))
        r1_ps = ctx.enter_context(tc.tile_pool(name="r1_ps", bufs=2, space="PSUM"))
        r2_ps = ctx.enter_context(tc.tile_pool(name="r2_ps", bufs=2, space="PSUM"))
        r3_ps = ctx.enter_context(tc.tile_pool(name="r3_ps", bufs=2, space="PSUM"))

        # =================== ATTENTION ===================
        # khl layout: [ones(1); k_hi(64); k_lo(0:63)] so rows 0:65 double as
        # the aug matmul's lhsT (k_lo dim 63 is dropped: ~0.007 nats of noise).
        # qdup: [zeros(1); q_hi(64); q_hi(0:63)]; qaug: [-m(1); q_lo(64)].
        def pass1(h):
            khl_t = khl_p.tile([128, S], F16, tag="khl")
            if h == 0:
                for jj in range(c.NCH):
                    nc.sync.dma_start(khl_t[:, jj * c.CH:(jj + 1) * c.CH],
                                      khl_d[h, :, jj * c.CH:(jj + 1) * c.CH])
            else:
                nc.sync.dma_start(khl_t[:], khl_d[h])
            qdup_t = qd_p.tile([128, T], F16, tag="qdup")
            nc.sync.dma_start(qdup_t[:], qdup_d[h])
            qh_t = qh_p.tile([128, T], F16, tag="qh")
            nc.sync.dma_start(qh_t[:], qh_d[h])
            qaug_t = qaug_p.tile([65, T], F16, tag="qaug")
            nc.sync.dma_start(qaug_t[1:65, :], qlo_d[h * D:(h + 1) * D, :])
            vaug = vv_p.tile([128, c.KT * 65], BF16, tag="vaug")
            nc.gpsimd.dma_start(vaug[:], v_d[h])

            m_neg = sm_p.tile([128, c.QT], F16, tag="mneg")
            for qt in range(c.QT):
                mtmp = sm_p.tile([128, c.NCH], F32, tag="mtmp")
                for j in range(c.NCH):
                    ps = r1_ps.tile([128, c.CH], F32, tag="R1")
                    nc.tensor.matmul(
                        ps[:], qh_t[:, qt * 128:(qt + 1) * 128],
                        khl_t[:, j * c.CH:(j + 1) * c.CH],
                        start=True, stop=True)
                    nc.vector.reduce_max(mtmp[:, j:j + 1], ps[:], axis=AX)
                nc.vector.reduce_max(m_neg[:, qt:qt + 1], mtmp[:],
                                     axis=AX, negate=True)
            # flip -max [128,QT] -> qaug row 0 [1,T] (perm order)
            nc.sync.dma_start(qaug_t[0:1, :], m_neg[:])
            return khl_t, qdup_t, qaug_t, vaug

        def pass2(h, tiles):
            khl_t, qdup_t, qaug_t, vaug = tiles
            ovp = r3_ps.tile([65, T], F32, tag="R3")
            for tb in range(0, c.KT, c.P2B):
                p2 = r2_ps.tile([128, c.P2B, T], F32, tag="R2")
                for ti in range(c.P2B):
                    t = tb + ti
                    tsl = slice(t * 128, (t + 1) * 128)
                    nc.tensor.matmul(p2[:, ti, :], khl_t[:, tsl],
                                     qdup_t[:], start=True, stop=False)
                    nc.tensor.matmul(p2[:, ti, :], khl_t[:65, tsl],
                                     qaug_t[:], start=False, stop=True)
                attnT = attn_p.tile([128, c.P2B, T], BF16, tag="attnT")
                nc.scalar.activation(attnT[:], p2[:],
                                     AF.Exp, bias=0.0, scale=c.scale)
                for ti in range(c.P2B):
                    t = tb + ti
                    nc.tensor.matmul(
                        ovp[:], vaug[:, t * 65:(t + 1) * 65], attnT[:, ti, :],
                        start=(t == 0), stop=(t == c.KT - 1),
                        skip_group_check=True)

            # 1/Z broadcast and drain into packed fc input
            zrow = zi_p.tile([1, T], F32, tag="zrow")
            nc.vector.reciprocal(zrow[:], ovp[64:65, :])
            # ACT copy releases the ov PSUM ring ~4us earlier than the
            # zinv bounce; the scale then reads the SBUF copy
            ovcp = zi_p.tile([64, T], F32, tag="ovcp")
            nc.scalar.copy(ovcp[:], ovp[:64, :])
            zdr = zdr_p.tile([1, T], F32, tag="zdr")
            nc.sync.dma_start(zdr[:], zrow[:])
            zinv_b = zi_p.tile([64, T], F32, tag="zinv")
            zsrc = zdr[:]
            nc.sync.dma_start(
                zinv_b[:],
                bass.AP(tensor=zsrc.tensor, offset=zsrc.offset,
                        ap=[[0, 64]] + list(zsrc.ap[1:])))
            po = (h % 2) * 64
            nc.vector.scalar_tensor_tensor(
                ovT_pack[po:po + 64, h // 2, :], ovcp[:], 1.0, zinv_b[:],
                OP.bypass, OP.mult)

        # software pipeline: pass1 of head h+1 issues before pass2 of head
        # h, so PE never waits on the max->flip->qaug latency chain
        staged = pass1(0)
        for h in range(H):
            nxt = pass1(h + 1) if h + 1 < H else None
            pass2(h, staged)
            staged = nxt

        # =================== FC + LN1 + transpose(h) ===================
        # fc accumulators ride the R1 ring, so fc overlaps the attention drain
        for qt in range(c.QT):
            qsl = slice(qt * 128, (qt + 1) * 128)
            hpre = st_p.tile([128, E], F32, tag="hpre")
            nc.sync.dma_start(hpre[:], qnat_d[qsl, :])
            for ec in range(c.NEC):
                esl = slice(ec * c.EC, (ec + 1) * c.EC)
                aps = r1_ps.tile([128, c.EC], F32, tag="R1")
                for dt in range(c.ET - 1):
                    nc.tensor.matmul(aps[:], ovT_pack[:, dt, qsl],
                                     wfc_t[:, dt, esl],
                                     start=(dt == 0), stop=False)
                nc.tensor.matmul(aps[:], ovT_pack[:, c.ET - 1, qsl],
                                 wfc_t[:, c.ET - 1, esl],
                                 start=False, stop=True)
                nc.vector.scalar_tensor_tensor(
                    hpre[:, esl], aps[:], 1.0, hpre[:, esl],
                    OP.bypass, OP.add)

            hbf = hb_p.tile([128, E], BF16, tag="hbf")
            _layernorm(nc, st_p, hpre[:], h_sb[:, qt, :], g1_b, be1_b, eps_t, c,
                       identity=ln1_id, bf16_out=hbf[:])
            for et in range(c.ET):
                nc.sync.dma_start(hT_bf[:, et, qsl],
                                  hbf[:, et * 128:(et + 1) * 128],
                                  transpose=True)

        # =================== FFN1 (zt pairs on the R2 ring) ===================
        for zp in range(c.ZT // 2):
            zps = r2_ps.tile([128, 2, T], F32, tag="R2")
            w1ts = []
            for i in range(2):
                w1t = w_p.tile([128, E], BF16, tag="w1t", name=f"w1t_{zp}_{i}")
                nc.gpsimd.dma_start(w1t[:], w1_d[zp * 2 + i, :, :])
                w1ts.append(w1t)
            for half in range(2):
                hsl = slice(half * (T // 2), (half + 1) * (T // 2))
                for i in range(2):
                    for et in range(c.ET):
                        nc.tensor.matmul(zps[:, i, hsl],
                                         w1ts[i][:, et * 128:(et + 1) * 128],
                                         hT_bf[:, et, hsl],
                                         start=(et == 0), stop=(et == c.ET - 1),
                                         skip_group_check=True)
            for i in range(2):
                zt = zp * 2 + i
                nc.scalar.activation(z1rel[:, zt, :], zps[:, i, :], AF.Relu,
                                     bias=b1_t[:, zt:zt + 1], scale=1.0)

        # ======== FFN2 + LN2, query-pair-serial so LN2 inlines ========
        # w2 is streamed once per query pair (re-read 2x, ~8MB extra DMA)
        # so each pair finishes early enough for its LN2 to overlap the next
        for qp in range(c.QT // 2):
            x_tiles = [r2_ps.tile([128, 2, c.EC], F32, tag="R2",
                                  name=f"x2_{qp}_{qi}")
                       for qi in range(2)]
            for zt in range(c.ZT):
                w2t = w_p.tile([128, E], BF16, tag="w2t")
                nc.gpsimd.dma_start(w2t[:], w2_d[zt, :, :])
                for qi in range(2):
                    qt = qp * 2 + qi
                    qsl = slice(qt * 128, (qt + 1) * 128)
                    for ec in range(c.NEC):
                        esl = slice(ec * c.EC, (ec + 1) * c.EC)
                        nc.tensor.matmul(
                            x_tiles[qi][:, ec, :], z1rel[:, zt, qsl],
                            w2t[:, esl], start=(zt == 0), stop=False,
                            skip_group_check=True)
            for qi in range(2):
                qt = qp * 2 + qi
                qsl = slice(qt * 128, (qt + 1) * 128)
                xacc = st_p.tile([128, E], F32, tag="hpre")
                for ec in range(c.NEC):
                    esl = slice(ec * c.EC, (ec + 1) * c.EC)
                    nc.tensor.matmul(x_tiles[qi][:, ec, :], ones_bf[:, :128],
                                     b2_t[:, esl], start=False, stop=True,
                                     skip_group_check=True)
                    nc.vector.scalar_tensor_tensor(
                        xacc[:, esl], x_tiles[qi][:, ec, :], 1.0,
                        h_sb[:, qt, esl], OP.bypass, OP.add)
                outt = st_p.tile([128, E], F32, tag="ln_t1")
                _layernorm(nc, st_p, xacc[:], outt[:], g2_b, be2_b, eps_t, c,
                           identity=ln2_id)
                for i in range(2):
                    esl2 = slice(i * (E // 2), (i + 1) * (E // 2))
                    nc.sync.dma_start(out_d[qsl, esl2], outt[:, esl2])

    return nc


def _split_waits(nc, maxw=1):
    """walrus in this toolchain only accepts 1 sync-wait per instruction on
    several formats; move excess waits onto preceding same-engine NoOps."""
    ctr = 0
    for f in nc.m.functions:
        for bb in f.blocks:
            out = []
            for inst in bb.instructions:
                si = getattr(inst, "sync_info", None)
                if si is not None and si.on_wait and len(si.on_wait) > maxw:
                    waits = list(si.on_wait)
                    head, tail = waits[:-maxw], waits[-maxw:]
                    for i in range(0, len(head), maxw):
                        ctr += 1
                        out.append(mybir.InstNoOp(
                            name=f"waitsplit_{ctr}", engine=inst.engine,
                            ins=[], outs=[],
                            sync_info=mybir.SyncInfo(
                                on_wait=list(head[i:i + maxw]), on_update=[]),
                        ))
                    si.on_wait = tail
                out.append(inst)
            bb.instructions[:] = out


# ======================= host side =======================

def _host_weights(c: Cfg, inputs):
    """Core-independent weight preps (computed once per kernel() call)."""
    Wv = np.asarray(inputs["Wv"], np.float64)
    Wfc = np.asarray(inputs["Wfc"], np.float64)            # [E, E]
    W1 = np.asarray(inputs["W1"], np.float64)              # [FE*E, E]
    W2 = np.asarray(inputs["W2"], np.float64)              # [E, FE*E]
    E, H, D = c.E, c.H, c.D

    # Wfc_v[e, h*64+d] = sum_dd Wfc[e, h*64+dd] * Wv[dd, d]
    wfcv = np.empty((E, E), np.float64)
    for h in range(H):
        wfcv[:, h * D:(h + 1) * D] = Wfc[:, h * D:(h + 1) * D] @ Wv
    wfc_prep = np.ascontiguousarray(
        wfcv.T.reshape(c.ET, 128, E).transpose(1, 0, 2)).astype(ml_bf16())

    # w1_prep[zt, p, et*128 + z] = W1[zt*128+z, et*128+p]
    w1r = W1.reshape(c.ZT, 128, c.ET, 128)                 # [zt, z, et, p]
    w1_prep = np.ascontiguousarray(
        w1r.transpose(0, 3, 2, 1).reshape(c.ZT, 128, E)).astype(ml_bf16())

    # w2_prep[zt, p, e] = W2[e, zt*128+p]
    w2_prep = np.ascontiguousarray(W2.T.reshape(c.ZT, 128, E)).astype(ml_bf16())

    b1 = np.asarray(inputs["b1"], np.float32)
    return {
        "wfc": wfc_prep,
        "w1": w1_prep,
        "b1": np.ascontiguousarray(b1.reshape(c.ZT, 128).T),
        "w2": w2_prep,
        "b2": np.asarray(inputs["b2"], np.float32)[None, :].astype(ml_bf16()),
        "g1": np.asarray(inputs["ln1_g"], np.float32)[None, :],
        "be1": np.asarray(inputs["ln1_b"], np.float32)[None, :],
        "g2": np.asarray(inputs["ln2_g"], np.float32)[None, :],
        "be2": np.asarray(inputs["ln2_b"], np.float32)[None, :],
    }


def _host_batch(c: Cfg, inputs, b):
    """Per-batch preps shared by the 4 cores of a batch."""
    k = np.asarray(inputs["keys"][b], np.float32)
    v = np.asarray(inputs["values"][b], np.float32)
    q = np.asarray(inputs["queries"][b], np.float32)       # [S, E]
    Wq = np.asarray(inputs["Wq"], np.float64)
    Wk = np.asarray(inputs["Wk"], np.float64)
    H_, D_ = c.H, c.D

    k_hi = k.astype(np.float16)
    k_lo = (k - k_hi.astype(np.float32)).astype(np.float16)
    # khl row 0 doubles as the aug-matmul ones row; k_lo dim 63 is dropped
    khl = np.empty((c.H, 128, c.S), np.float16)
    for h in range(H_):
        khl[h, 0] = 1.0
        khl[h, 1:65] = k_hi[:, h * D_:(h + 1) * D_].T
        khl[h, 65:] = k_lo[:, h * D_:h * D_ + 63].T

    # pretiled v with ones column: v_prep[h, p, t*65+d] = v[t*128+p, h*64+d]
    vt = v.reshape(c.KT, 128, c.H, c.D).astype(ml_bf16())  # [t, p, h, d]
    v_prep = np.ones((c.H, 128, c.KT, 65), ml_bf16())
    v_prep[:, :, :, :64] = vt.transpose(2, 1, 0, 3)
    v_prep = v_prep.reshape(c.H, 128, c.KT * 65)

    # fold Wq/Wk into the queries: q' = q @ (Wq.T @ Wk) per head
    A_mid = Wq.T @ Wk
    qp = np.empty((c.S, c.E), np.float32)
    for h in range(H_):
        qp[:, h * D_:(h + 1) * D_] = (
            q[:, h * D_:(h + 1) * D_].astype(np.float64) @ A_mid
        ).astype(np.float32)
    return {"khl": khl, "vv": v_prep, "qp": qp, "q": q}


_HOST_CACHE = {}


def host_prep(c: Cfg, inputs, core):
    """Build the per-core input map (numpy only; shared preps cached)."""
    B = inputs["queries"].shape[0]
    cores_per_batch = 8 // B if B <= 8 else 1
    b = core // cores_per_batch
    slot = core % cores_per_batch
    T = c.T
    perm = c.perm()
    H_, D_ = c.H, c.D

    if _HOST_CACHE.get("token") is not inputs:
        _HOST_CACHE.clear()
        _HOST_CACHE["token"] = inputs
    if "w" not in _HOST_CACHE:
        _HOST_CACHE["w"] = _host_weights(c, inputs)
    if ("b", b) not in _HOST_CACHE:
        _HOST_CACHE[("b", b)] = _host_batch(c, inputs, b)
    w = _HOST_CACHE["w"]
    bt = _HOST_CACHE[("b", b)]

    qp = bt["qp"][slot * T:(slot + 1) * T]                 # [T, E] fp32
    qs = bt["q"][slot * T:(slot + 1) * T]                  # [T, E]
    q_hi = qp.astype(np.float16)
    q_lo = (qp - q_hi.astype(np.float32)).astype(np.float16)

    # pass1 lhsT padded to 128 rows: zeros align with khl's ones/k_lo rows
    qh_pad = np.zeros((c.H, 128, T), np.float16)
    for h in range(H_):
        qh_pad[h, 1:65] = q_hi[:, h * D_:(h + 1) * D_].T
    qhp = q_hi[perm]                                       # [T, E] perm order
    qdup = np.empty((c.H, 128, T), np.float16)
    for h in range(H_):
        qdup[h, 0] = 0.0
        qdup[h, 1:65] = qhp[:, h * D_:(h + 1) * D_].T
        qdup[h, 65:] = qhp[:, h * D_:h * D_ + 63].T
    qlo_ship = np.ascontiguousarray(q_lo[perm].T)          # [E, T] perm order

    ret = {
        "khl": bt["khl"],
        "vv": bt["vv"],
        "qdup": qdup,
        "qh": qh_pad,
        "qlo": qlo_ship,
        "qnat": np.ascontiguousarray(
            qs[perm] + np.asarray(inputs["bfc"], np.float32)[None, :]),
    }
    ret.update(w)
    return ret


def ml_bf16():
    import ml_dtypes
    return ml_dtypes.bfloat16


_CACHE = {}


def kernel(**inputs):
    """Full-input entry point: shard across 8 cores, run, gather."""
    c = Cfg()
    B, S, E = inputs["queries"].shape
    assert (B, S, E) == (2, c.S, c.E), (B, S, E)

    ln1_id = bool(np.all(np.asarray(inputs["ln1_g"]) == 1.0)
                  and np.all(np.asarray(inputs["ln1_b"]) == 0.0))
    ln2_id = bool(np.all(np.asarray(inputs["ln2_g"]) == 1.0)
                  and np.all(np.asarray(inputs["ln2_b"]) == 0.0))
    key = ("nc", ln1_id, ln2_id)
    if key not in _CACHE:
        nc = build_nc(c, ln1_id, ln2_id)
        _split_waits(nc)   # walrus wait-slot workaround (compile path only)
        _CACHE[key] = nc
    _CACHE["nc"] = nc = _CACHE[key]

    in_maps = [host_prep(c, inputs, core) for core in range(8)]

    from concourse.bass_utils import run_bass_kernel_spmd
    res = run_bass_kernel_spmd(nc, in_maps, core_ids=list(range(8)))

    perm = c.perm()
    out = np.empty((B, S, E), np.float32)
    cores_per_batch = 4
    for core in range(8):
        b = core // cores_per_batch
        slot = core % cores_per_batch
        block = np.empty((c.T, E), np.float32)
        block[perm] = res.results[core]["out"]
        out[b, slot * c.T:(slot + 1) * c.T] = block
    return out


# revision 38
# speedup vs baseline: 1.0133x; 1.0030x over previous
"""Trainium2 Bass kernel for nn_Encoder_Block (B=2,S=2048,E=1024,H=16,D=64,FE=4).

Sharding: 8 NeuronCores, no collectives. Cores 0-3 take batch 0, cores 4-7
batch 1; each core owns a 512-query slice and runs the full encoder block
for those queries (it loads all keys/values of its batch plus all weights).

Attention math (per head, per core; scores need ~20+ mantissa bits because the
reference multiplies logits by sqrt(S)=45.25, so bf16/fp16 single products
flip argmaxes of the near-one-hot softmax):
  q' = q @ (Wq^T Wk) folded on host; split q' = q_hi + q_lo, k = k_hi + k_lo
  in fp16 (11-bit mantissa each; products are exact in fp32 PSUM).
  pass1 [q,k]: s_hi = q_hi.k_hi -> row max via DVE tensor_tensor_reduce pairs
               over PSUM chunks -> -max as fp16 [128,QT]
  flip: one SBUF->SBUF DMA turns -max [128,QT] into the [1,T] aug row of qaug
        (query perm order j = r*QT + qt makes that DMA a linear copy)
  pass2 [k,q] per k-tile: psum = [k_hi;k_lo].[q_hi;q_hi] + [k_hi;ones].[q_lo;-m]
        = k.q_hi + k_hi.q_lo - max   (drops only k_lo.q_lo ~ 2^-24)
  exp via ACT (scale=sqrt(S)) -> attnT bf16 -> ov: [v|1].T @ attnT accumulated
  over k-tiles gives [ovT; Z]; 1/Z broadcast multiplies during the drain
  straight into the packed fc input (Wv folded into Wfc on host).
Then fc + residual + LN1, FFN1(+relu, bias), FFN2(+bias), residual + LN2.
v is host-pretiled with the ones column baked in; weight/value DMAs ride the
GPSIMD SWDGE queue to keep the HWDGE path clear for the latency-critical
attention streams.
"""
import os
import sys
import math
from contextlib import ExitStack

os.environ.setdefault("NEURON_RT_RESET_CORES", "1")
sys.path.insert(0, "/opt/trn_rl_repo")

import numpy as np
import concourse.bass as bass
import concourse.tile as tile
from concourse import mybir

F32 = mybir.dt.float32
F16 = mybir.dt.float16
BF16 = mybir.dt.bfloat16
AX = mybir.AxisListType.X
AF = mybir.ActivationFunctionType
OP = mybir.AluOpType

class Cfg:
    def __init__(self, S=2048, E=1024, H=16, D=64, FE=4, T=512, eps=1e-5):
        self.S, self.E, self.H, self.D, self.FE, self.T, self.eps = S, E, H, D, FE, T, eps
        assert D == 64 and E == H * D
        self.KT = S // 128            # k partition-tiles
        self.QT = T // 128            # q tiles (per core)
        self.ET = E // 128            # e tiles
        self.ZT = FE * E // 128       # ffn hidden tiles
        self.CH = min(512, S)         # k moving chunk for pass1
        self.NCH = S // self.CH
        self.EC = min(512, E)         # e moving chunk
        self.NEC = E // self.EC
        self.P2B = 2                  # pass-2 k-tiles per exp batch
        self.scale = math.sqrt(float(S))

    def perm(self):
        # pass-2 query order j <-> original query (j % QT)*128 + j // QT
        j = np.arange(self.T)
        return (j % self.QT) * 128 + j // self.QT


def _layernorm(nc, pool, x_ap, out_ap, g_b, b_b, eps_t, c,
               identity=False, bf16_out=None):
    """LayerNorm over the free dim (E) of x_ap [128, E] -> out_ap.

    The affine (x - mu) * rstd runs on ACT (Identity: per-partition scale and
    bias APs, same table as Exp/Relu so no table reload); DVE keeps the stats
    and the per-free-element gamma/beta ops. With identity=True (gamma==1,
    beta==0, detected from the actual inputs at build time) the gamma/beta
    DVE ops are skipped and ACT writes out_ap (and optionally a bf16 twin)
    directly."""
    E = c.E
    nsub = (E + 511) // 512
    stats = pool.tile([128, nsub, 6], F32, tag="ln_stats")
    xr = x_ap.rearrange("p (n s) -> p n s", n=nsub)
    for i in range(nsub):
        nc.vector.bn_stats(stats[:, i, :], xr[:, i, :])
    mv = pool.tile([128, 2], F32, tag="ln_mv")
    nc.vector.bn_aggr(mv[:], stats[:])
    rstd = pool.tile([128, 1], F32, tag="ln_rstd")
    nc.scalar.activation(rstd[:], mv[:, 1:2], AF.Sqrt, bias=eps_t[:], scale=1.0)
    nc.vector.reciprocal(rstd[:], rstd[:])
    nmr = pool.tile([128, 1], F32, tag="ln_nmr")
    nc.vector.scalar_tensor_tensor(
        nmr[:], mv[:, 0:1], -1.0, rstd[:], OP.mult, OP.mult)
    if identity:
        if bf16_out is not None:
            nc.scalar.activation(bf16_out, x_ap, AF.Identity,
                                 bias=nmr[:], scale=rstd[:])
            nc.scalar.activation(out_ap, x_ap, AF.Identity,
                                 bias=nmr[:], scale=rstd[:])
            return
        half = E // 2
        for i in range(2):
            nc.scalar.activation(out_ap[:, i * half:(i + 1) * half],
                                 x_ap[:, i * half:(i + 1) * half],
                                 AF.Identity, bias=nmr[:], scale=rstd[:])
        return
    t1 = pool.tile([128, E], F32, tag="ln_t1")
    nc.scalar.activation(t1[:], x_ap, AF.Identity, bias=nmr[:], scale=rstd[:])
    if bf16_out is not None:
        nc.vector.tensor_tensor(t1[:], t1[:], g_b[:], OP.mult)
        nc.vector.tensor_tensor(out_ap, t1[:], b_b[:], OP.add)
        nc.scalar.copy(bf16_out, out_ap)
        return
    nc.vector.tensor_tensor(t1[:], t1[:], g_b[:], OP.mult)
    nc.vector.tensor_tensor(out_ap, t1[:], b_b[:], OP.add)


def build_nc(c: Cfg, ln1_id=False, ln2_id=False):
    """Build the single-core program (pure SPMD — all cores run this)."""
    nc = bass.Bass()
    S, E, H, D, T = c.S, c.E, c.H, c.D, c.T

    dp = nc.declare_dram_parameter
    khl_d = dp("khl", [H, 128, S], F16, isOutput=False)      # [1; k_hi^T; k_lo^T[:63]]
    qdup_d = dp("qdup", [H, 128, T], F16, isOutput=False)    # [0; q_hi; q_hi[:63]] perm
    qh_d = dp("qh", [H, 128, T], F16, isOutput=False)        # [0; q_hi^T; 0] orig order
    qlo_d = dp("qlo", [E, T], F16, isOutput=False)           # q_lo^T perm order
    qnat_d = dp("qnat", [T, E], F32, isOutput=False)         # queries rows (perm order)
    v_d = dp("vv", [H, 128, c.KT * 65], BF16, isOutput=False)  # pretiled v + ones col
    wfc_d = dp("wfc", [128, c.ET, E], BF16, isOutput=False)  # Wfc_v^T tiled
    w1_d = dp("w1", [c.ZT, 128, E], BF16, isOutput=False)    # per zt: [e_in part, z cols]
    b1_d = dp("b1", [128, c.ZT], F32, isOutput=False)
    w2_d = dp("w2", [c.ZT, 128, E], BF16, isOutput=False)    # per zt: [z part, e cols]
    b2_d = dp("b2", [1, E], BF16, isOutput=False)
    g1_d = dp("g1", [1, E], F32, isOutput=False)
    be1_d = dp("be1", [1, E], F32, isOutput=False)
    g2_d = dp("g2", [1, E], F32, isOutput=False)
    be2_d = dp("be2", [1, E], F32, isOutput=False)
    out_d = dp("out", [T, E], F32, isOutput=True)            # perm rows

    with tile.TileContext(nc) as tc, ExitStack() as ctx:
        persist = ctx.enter_context(tc.tile_pool(name="persist", bufs=1))

        def bcast128(src_ap, nm, dtype=BF16):
            t = persist.tile([128, src_ap.shape[1]], dtype, name=nm, tag=nm)
            src_b = bass.AP(tensor=src_ap.tensor, offset=src_ap.offset,
                            ap=[[0, 128]] + list(src_ap.ap[1:]))
            nc.gpsimd.dma_start(t[:], src_b)
            return t

        g1_b = bcast128(g1_d[:], "g1b")
        be1_b = bcast128(be1_d[:], "be1b")
        g2_b = bcast128(g2_d[:], "g2b")
        be2_b = bcast128(be2_d[:], "be2b")

        eps_t = persist.tile([128, 1], F32)
        nc.vector.memset(eps_t[:], c.eps)

        ones_bf = persist.tile([1, 128], BF16)
        nc.vector.memset(ones_bf[:], 1.0)

        # staged via the GPSIMD SWDGE queue: first use is after attention, and
        # this keeps the HWDGE path clear for head 0's latency-critical streams
        wfc_t = persist.tile([128, c.ET, E], BF16)
        nc.gpsimd.dma_start(wfc_t[:], wfc_d[:])
        b1_t = persist.tile([128, c.ZT], F32)
        nc.gpsimd.dma_start(b1_t[:], b1_d[:])
        b2_t = persist.tile([1, E], BF16)
        nc.gpsimd.dma_start(b2_t[:], b2_d[:])

        ovT_pack = persist.tile([128, c.ET, T], BF16)
        h_sb = persist.tile([128, c.QT, E], F32)
        hT_bf = persist.tile([128, c.ET, T], BF16)
        z1rel = persist.tile([128, c.ZT, T], BF16)

        # ---- flat pools; PSUM = three shared rings (2+4+2 banks), reused
        # across attention/fc/FFN so phases overlap with no scope barriers ----
        khl_p = ctx.enter_context(tc.tile_pool(name="khl", bufs=3))
        qd_p = ctx.enter_context(tc.tile_pool(name="qd", bufs=3))
        qh_p = ctx.enter_context(tc.tile_pool(name="qh", bufs=3))
        qaug_p = ctx.enter_context(tc.tile_pool(name="qaug", bufs=3))
        vv_p = ctx.enter_context(tc.tile_pool(name="vv", bufs=3))
        sm_p = ctx.enter_context(tc.tile_pool(name="sm", bufs=3))
        attn_p = ctx.enter_context(tc.tile_pool(name="attn", bufs=4))
        hb_p = ctx.enter_context(tc.tile_pool(name="hb", bufs=2))
        zi_p = ctx.enter_context(tc.tile_pool(name="zi", bufs=3))
        zdr_p = ctx.enter_context(tc.tile_pool(name="zdr", bufs=3, space="DRAM"))
        st_p = ctx.enter_context(tc.tile_pool(name="st", bufs=3))
        w_p = ctx.enter_context(tc.tile_pool(name="wstream", bufs=6))
        r1_ps = ctx.enter_context(tc.tile_pool(name="r1_ps", bufs=2, space="PSUM"))
        r2_ps = ctx.enter_context(tc.tile_pool(name="r2_ps", bufs=2, space="PSUM"))
        r3_ps = ctx.enter_context(tc.tile_pool(name="r3_ps", bufs=2, space="PSUM"))

        # =================== ATTENTION ===================
        # khl layout: [ones(1); k_hi(64); k_lo(0:63)] so rows 0:65 double as
        # the aug matmul's lhsT (k_lo dim 63 is dropped: ~0.007 nats of noise).
        # qdup: [zeros(1); q_hi(64); q_hi(0:63)]; qaug: [-m(1); q_lo(64)].
        def pass1(h):
            khl_t = khl_p.tile([128, S], F16, tag="khl")
            if h <= 1:
                for jj in range(c.NCH):
                    nc.sync.dma_start(khl_t[:, jj * c.CH:(jj + 1) * c.CH],
                                      khl_d[h, :, jj * c.CH:(jj + 1) * c.CH])
            else:
                nc.sync.dma_start(khl_t[:], khl_d[h])
            qdup_t = qd_p.tile([128, T], F16, tag="qdup")
            nc.sync.dma_start(qdup_t[:], qdup_d[h])
            qh_t = qh_p.tile([128, T], F16, tag="qh")
            nc.sync.dma_start(qh_t[:], qh_d[h])
            qaug_t = qaug_p.tile([65, T], F16, tag="qaug")
            nc.sync.dma_start(qaug_t[1:65, :], qlo_d[h * D:(h + 1) * D, :])
            vaug = vv_p.tile([128, c.KT * 65], BF16, tag="vaug")
            nc.gpsimd.dma_start(vaug[:], v_d[h])

            m_neg = sm_p.tile([128, c.QT], F16, tag="mneg")
            for qt in range(c.QT):
                mtmp = sm_p.tile([128, c.NCH], F32, tag="mtmp")
                for j in range(c.NCH):
                    ps = r1_ps.tile([128, c.CH], F32, tag="R1")
                    nc.tensor.matmul(
                        ps[:], qh_t[:, qt * 128:(qt + 1) * 128],
                        khl_t[:, j * c.CH:(j + 1) * c.CH],
                        start=True, stop=True)
                    nc.vector.reduce_max(mtmp[:, j:j + 1], ps[:], axis=AX)
                nc.vector.reduce_max(m_neg[:, qt:qt + 1], mtmp[:],
                                     axis=AX, negate=True)
            # flip -max [128,QT] -> qaug row 0 [1,T] (perm order)
            nc.sync.dma_start(qaug_t[0:1, :], m_neg[:])
            return khl_t, qdup_t, qaug_t, vaug

        def pass2(h, tiles):
            khl_t, qdup_t, qaug_t, vaug = tiles
            ovp = r3_ps.tile([65, T], F32, tag="R3")
            for tb in range(0, c.KT, c.P2B):
                p2 = r2_ps.tile([128, c.P2B, T], F32, tag="R2")
                for ti in range(c.P2B):
                    t = tb + ti
                    tsl = slice(t * 128, (t + 1) * 128)
                    nc.tensor.matmul(p2[:, ti, :], khl_t[:, tsl],
                                     qdup_t[:], start=True, stop=False)
                    nc.tensor.matmul(p2[:, ti, :], khl_t[:65, tsl],
                                     qaug_t[:], start=False, stop=True)
                attnT = attn_p.tile([128, c.P2B, T], BF16, tag="attnT")
                nc.scalar.activation(attnT[:], p2[:],
                                     AF.Exp, bias=0.0, scale=c.scale)
                for ti in range(c.P2B):
                    t = tb + ti
                    nc.tensor.matmul(
                        ovp[:], vaug[:, t * 65:(t + 1) * 65], attnT[:, ti, :],
                        start=(t == 0), stop=(t == c.KT - 1),
                        skip_group_check=True)

            # 1/Z broadcast and drain into packed fc input
            zrow = zi_p.tile([1, T], F32, tag="zrow")
            nc.vector.reciprocal(zrow[:], ovp[64:65, :])
            # ACT copy releases the ov PSUM ring ~4us earlier than the
            # zinv bounce; the scale then reads the SBUF copy
            ovcp = zi_p.tile([64, T], F32, tag="ovcp")
            nc.scalar.copy(ovcp[:], ovp[:64, :])
            zdr = zdr_p.tile([1, T], F32, tag="zdr")
            nc.sync.dma_start(zdr[:], zrow[:])
            zinv_b = zi_p.tile([64, T], F32, tag="zinv")
            zsrc = zdr[:]
            nc.sync.dma_start(
                zinv_b[:],
                bass.AP(tensor=zsrc.tensor, offset=zsrc.offset,
                        ap=[[0, 64]] + list(zsrc.ap[1:])))
            po = (h % 2) * 64
            nc.vector.scalar_tensor_tensor(
                ovT_pack[po:po + 64, h // 2, :], ovcp[:], 1.0, zinv_b[:],
                OP.bypass, OP.mult)

        # software pipeline: pass1 of head h+1 issues before pass2 of head
        # h, so PE never waits on the max->flip->qaug latency chain
        staged = pass1(0)
        for h in range(H):
            nxt = pass1(h + 1) if h + 1 < H else None
            pass2(h, staged)
            staged = nxt

        # =================== FC + LN1 + transpose(h) ===================
        # fc accumulators ride the R1 ring, so fc overlaps the attention drain
        for qt in range(c.QT):
            qsl = slice(qt * 128, (qt + 1) * 128)
            hpre = st_p.tile([128, E], F32, tag="hpre")
            nc.sync.dma_start(hpre[:], qnat_d[qsl, :])
            for ec in range(c.NEC):
                esl = slice(ec * c.EC, (ec + 1) * c.EC)
                aps = r1_ps.tile([128, c.EC], F32, tag="R1")
                for dt in range(c.ET - 1):
                    nc.tensor.matmul(aps[:], ovT_pack[:, dt, qsl],
                                     wfc_t[:, dt, esl],
                                     start=(dt == 0), stop=False)
                nc.tensor.matmul(aps[:], ovT_pack[:, c.ET - 1, qsl],
                                 wfc_t[:, c.ET - 1, esl],
                                 start=False, stop=True)
                nc.vector.scalar_tensor_tensor(
                    hpre[:, esl], aps[:], 1.0, hpre[:, esl],
                    OP.bypass, OP.add)

            hbf = hb_p.tile([128, E], BF16, tag="hbf")
            _layernorm(nc, st_p, hpre[:], h_sb[:, qt, :], g1_b, be1_b, eps_t, c,
                       identity=ln1_id, bf16_out=hbf[:])
            for et in range(c.ET):
                nc.sync.dma_start(hT_bf[:, et, qsl],
                                  hbf[:, et * 128:(et + 1) * 128],
                                  transpose=True)

        # =================== FFN1 (zt pairs on the R2 ring) ===================
        for zp in range(c.ZT // 2):
            zps = r2_ps.tile([128, 2, T], F32, tag="R2")
            w1ts = []
            for i in range(2):
                w1t = w_p.tile([128, E], BF16, tag="w1t", name=f"w1t_{zp}_{i}")
                nc.gpsimd.dma_start(w1t[:], w1_d[zp * 2 + i, :, :])
                w1ts.append(w1t)
            for half in range(2):
                hsl = slice(half * (T // 2), (half + 1) * (T // 2))
                for i in range(2):
                    for et in range(c.ET):
                        nc.tensor.matmul(zps[:, i, hsl],
                                         w1ts[i][:, et * 128:(et + 1) * 128],
                                         hT_bf[:, et, hsl],
                                         start=(et == 0), stop=(et == c.ET - 1),
                                         skip_group_check=True)
            for i in range(2):
                zt = zp * 2 + i
                nc.scalar.activation(z1rel[:, zt, :], zps[:, i, :], AF.Relu,
                                     bias=b1_t[:, zt:zt + 1], scale=1.0)

        # ======== FFN2 + LN2, query-pair-serial so LN2 inlines ========
        # w2 is streamed once per query pair (re-read 2x, ~8MB extra DMA)
        # so each pair finishes early enough for its LN2 to overlap the next
        for qp in range(c.QT // 2):
            x_tiles = [r2_ps.tile([128, 2, c.EC], F32, tag="R2",
                                  name=f"x2_{qp}_{qi}")
                       for qi in range(2)]
            for zt in range(c.ZT):
                w2t = w_p.tile([128, E], BF16, tag="w2t")
                nc.gpsimd.dma_start(w2t[:], w2_d[zt, :, :])
                for qi in range(2):
                    qt = qp * 2 + qi
                    qsl = slice(qt * 128, (qt + 1) * 128)
                    for ec in range(c.NEC):
                        esl = slice(ec * c.EC, (ec + 1) * c.EC)
                        nc.tensor.matmul(
                            x_tiles[qi][:, ec, :], z1rel[:, zt, qsl],
                            w2t[:, esl], start=(zt == 0), stop=False,
                            skip_group_check=True)
            for qi in range(2):
                qt = qp * 2 + qi
                qsl = slice(qt * 128, (qt + 1) * 128)
                xacc = st_p.tile([128, E], F32, tag="hpre")
                for ec in range(c.NEC):
                    esl = slice(ec * c.EC, (ec + 1) * c.EC)
                    nc.tensor.matmul(x_tiles[qi][:, ec, :], ones_bf[:, :128],
                                     b2_t[:, esl], start=False, stop=True,
                                     skip_group_check=True)
                    nc.vector.scalar_tensor_tensor(
                        xacc[:, esl], x_tiles[qi][:, ec, :], 1.0,
                        h_sb[:, qt, esl], OP.bypass, OP.add)
                outt = st_p.tile([128, E], F32, tag="ln_t1")
                _layernorm(nc, st_p, xacc[:], outt[:], g2_b, be2_b, eps_t, c,
                           identity=ln2_id)
                for i in range(2):
                    esl2 = slice(i * (E // 2), (i + 1) * (E // 2))
                    nc.sync.dma_start(out_d[qsl, esl2], outt[:, esl2])

    return nc


def _split_waits(nc, maxw=1):
    """walrus in this toolchain only accepts 1 sync-wait per instruction on
    several formats; move excess waits onto preceding same-engine NoOps."""
    ctr = 0
    for f in nc.m.functions:
        for bb in f.blocks:
            out = []
            for inst in bb.instructions:
                si = getattr(inst, "sync_info", None)
                if si is not None and si.on_wait and len(si.on_wait) > maxw:
                    waits = list(si.on_wait)
                    head, tail = waits[:-maxw], waits[-maxw:]
                    for i in range(0, len(head), maxw):
                        ctr += 1
                        out.append(mybir.InstNoOp(
                            name=f"waitsplit_{ctr}", engine=inst.engine,
                            ins=[], outs=[],
                            sync_info=mybir.SyncInfo(
                                on_wait=list(head[i:i + maxw]), on_update=[]),
                        ))
                    si.on_wait = tail
                out.append(inst)
            bb.instructions[:] = out


# ======================= host side =======================

def _host_weights(c: Cfg, inputs):
    """Core-independent weight preps (computed once per kernel() call)."""
    Wv = np.asarray(inputs["Wv"], np.float64)
    Wfc = np.asarray(inputs["Wfc"], np.float64)            # [E, E]
    W1 = np.asarray(inputs["W1"], np.float64)              # [FE*E, E]
    W2 = np.asarray(inputs["W2"], np.float64)              # [E, FE*E]
    E, H, D = c.E, c.H, c.D

    # Wfc_v[e, h*64+d] = sum_dd Wfc[e, h*64+dd] * Wv[dd, d]
    wfcv = np.empty((E, E), np.float64)
    for h in range(H):
        wfcv[:, h * D:(h + 1) * D] = Wfc[:, h * D:(h + 1) * D] @ Wv
    wfc_prep = np.ascontiguousarray(
        wfcv.T.reshape(c.ET, 128, E).transpose(1, 0, 2)).astype(ml_bf16())

    # w1_prep[zt, p, et*128 + z] = W1[zt*128+z, et*128+p]
    w1r = W1.reshape(c.ZT, 128, c.ET, 128)                 # [zt, z, et, p]
    w1_prep = np.ascontiguousarray(
        w1r.transpose(0, 3, 2, 1).reshape(c.ZT, 128, E)).astype(ml_bf16())

    # w2_prep[zt, p, e] = W2[e, zt*128+p]
    w2_prep = np.ascontiguousarray(W2.T.reshape(c.ZT, 128, E)).astype(ml_bf16())

    b1 = np.asarray(inputs["b1"], np.float32)
    return {
        "wfc": wfc_prep,
        "w1": w1_prep,
        "b1": np.ascontiguousarray(b1.reshape(c.ZT, 128).T),
        "w2": w2_prep,
        "b2": np.asarray(inputs["b2"], np.float32)[None, :].astype(ml_bf16()),
        "g1": np.asarray(inputs["ln1_g"], np.float32)[None, :],
        "be1": np.asarray(inputs["ln1_b"], np.float32)[None, :],
        "g2": np.asarray(inputs["ln2_g"], np.float32)[None, :],
        "be2": np.asarray(inputs["ln2_b"], np.float32)[None, :],
    }


def _host_batch(c: Cfg, inputs, b):
    """Per-batch preps shared by the 4 cores of a batch."""
    k = np.asarray(inputs["keys"][b], np.float32)
    v = np.asarray(inputs["values"][b], np.float32)
    q = np.asarray(inputs["queries"][b], np.float32)       # [S, E]
    Wq = np.asarray(inputs["Wq"], np.float64)
    Wk = np.asarray(inputs["Wk"], np.float64)
    H_, D_ = c.H, c.D

    k_hi = k.astype(np.float16)
    k_lo = (k - k_hi.astype(np.float32)).astype(np.float16)
    # khl row 0 doubles as the aug-matmul ones row; k_lo dim 63 is dropped
    khl = np.empty((c.H, 128, c.S), np.float16)
    for h in range(H_):
        khl[h, 0] = 1.0
        khl[h, 1:65] = k_hi[:, h * D_:(h + 1) * D_].T
        khl[h, 65:] = k_lo[:, h * D_:h * D_ + 63].T

    # pretiled v with ones column: v_prep[h, p, t*65+d] = v[t*128+p, h*64+d]
    vt = v.reshape(c.KT, 128, c.H, c.D).astype(ml_bf16())  # [t, p, h, d]
    v_prep = np.ones((c.H, 128, c.KT, 65), ml_bf16())
    v_prep[:, :, :, :64] = vt.transpose(2, 1, 0, 3)
    v_prep = v_prep.reshape(c.H, 128, c.KT * 65)

    # fold Wq/Wk into the queries: q' = q @ (Wq.T @ Wk) per head
    A_mid = Wq.T @ Wk
    qp = np.empty((c.S, c.E), np.float32)
    for h in range(H_):
        qp[:, h * D_:(h + 1) * D_] = (
            q[:, h * D_:(h + 1) * D_].astype(np.float64) @ A_mid
        ).astype(np.float32)
    return {"khl": khl, "vv": v_prep, "qp": qp, "q": q}


_HOST_CACHE = {}


def host_prep(c: Cfg, inputs, core):
    """Build the per-core input map (numpy only; shared preps cached)."""
    B = inputs["queries"].shape[0]
    cores_per_batch = 8 // B if B <= 8 else 1
    b = core // cores_per_batch
    slot = core % cores_per_batch
    T = c.T
    perm = c.perm()
    H_, D_ = c.H, c.D

    if _HOST_CACHE.get("token") is not inputs:
        _HOST_CACHE.clear()
        _HOST_CACHE["token"] = inputs
    if "w" not in _HOST_CACHE:
        _HOST_CACHE["w"] = _host_weights(c, inputs)
    if ("b", b) not in _HOST_CACHE:
        _HOST_CACHE[("b", b)] = _host_batch(c, inputs, b)
    w = _HOST_CACHE["w"]
    bt = _HOST_CACHE[("b", b)]

    qp = bt["qp"][slot * T:(slot + 1) * T]                 # [T, E] fp32
    qs = bt["q"][slot * T:(slot + 1) * T]                  # [T, E]
    q_hi = qp.astype(np.float16)
    q_lo = (qp - q_hi.astype(np.float32)).astype(np.float16)

    # pass1 lhsT padded to 128 rows: zeros align with khl's ones/k_lo rows
    qh_pad = np.zeros((c.H, 128, T), np.float16)
    for h in range(H_):
        qh_pad[h, 1:65] = q_hi[:, h * D_:(h + 1) * D_].T
    qhp = q_hi[perm]                                       # [T, E] perm order
    qdup = np.empty((c.H, 128, T), np.float16)
    for h in range(H_):
        qdup[h, 0] = 0.0
        qdup[h, 1:65] = qhp[:, h * D_:(h + 1) * D_].T
        qdup[h, 65:] = qhp[:, h * D_:h * D_ + 63].T
    qlo_ship = np.ascontiguousarray(q_lo[perm].T)          # [E, T] perm order

    ret = {
        "khl": bt["khl"],
        "vv": bt["vv"],
        "qdup": qdup,
        "qh": qh_pad,
        "qlo": qlo_ship,
        "qnat": np.ascontiguousarray(
            qs[perm] + np.asarray(inputs["bfc"], np.float32)[None, :]),
    }
    ret.update(w)
    return ret


def ml_bf16():
    import ml_dtypes
    return ml_dtypes.bfloat16


_CACHE = {}


def kernel(**inputs):
    """Full-input entry point: shard across 8 cores, run, gather."""
    c = Cfg()
    B, S, E = inputs["queries"].shape
    assert (B, S, E) == (2, c.S, c.E), (B, S, E)

    ln1_id = bool(np.all(np.asarray(inputs["ln1_g"]) == 1.0)
                  and np.all(np.asarray(inputs["ln1_b"]) == 0.0))
    ln2_id = bool(np.all(np.asarray(inputs["ln2_g"]) == 1.0)
                  and np.all(np.asarray(inputs["ln2_b"]) == 0.0))
    key = ("nc", ln1_id, ln2_id)
    if key not in _CACHE:
        nc = build_nc(c, ln1_id, ln2_id)
        _split_waits(nc)   # walrus wait-slot workaround (compile path only)
        _CACHE[key] = nc
    _CACHE["nc"] = nc = _CACHE[key]

    in_maps = [host_prep(c, inputs, core) for core in range(8)]

    from concourse.bass_utils import run_bass_kernel_spmd
    res = run_bass_kernel_spmd(nc, in_maps, core_ids=list(range(8)))

    perm = c.perm()
    out = np.empty((B, S, E), np.float32)
    cores_per_batch = 4
    for core in range(8):
        b = core // cores_per_batch
        slot = core % cores_per_batch
        block = np.empty((c.T, E), np.float32)
        block[perm] = res.results[core]["out"]
        out[b, slot * c.T:(slot + 1) * c.T] = block
    return out
